# revision 21
# baseline (speedup 1.0000x reference)
"""Trainium2 Bass kernel for nn_AVGAE (3-layer GAT variational graph
autoencoder, N=4096) on 8 NeuronCores.

Sharding: 1D row partition of the N x N attention/score matrices — core k
owns output rows [512k, 512k+512). Small per-node features are all-gathered
between layers (AllGather over internal DRAM tiles).

Key algebraic restructuring (no elementwise transcendentals over N x N):
  exp(leaky_relu(f1_i + f2_j, a)) = max(A_i*B_j, C_i*D_j)
  with A=exp(f1), B=exp(f2), C=exp(a*f1), D=exp(a*f2)
so each N x N score tile is built with vector ALU ops only (outer-product
scalar muls + max + mask mul), all bf16, then consumed directly by the
tensor engine as attention weights.  Softmax denominators come for free as
a ones-column in the attention rhs (exp(MASK_VAL) == 0 exactly in fp32, so
masked entries contribute 0 to numerator and denominator, matching the
reference softmax).

All per-node "h" quantities of layers 1/2 are linear images of layer-0
attention output, so the layer-0 attention rhs carries
[h0@W1 | h0@W2 | per-layer score vectors | ones] and hidden itself is never
materialized.  Host precomputes the folded weight matrix.  Every core
builds the FULL 4096-row layer-0 rhs G locally from the full X (f16) —
~15us of redundant PE beats the ~25us fixed latency + skew barrier of an
AllGather (collectives here cost ~25us regardless of payload size, so the
kernel keeps exactly two: the g1 feature gather and the Z^T gather).

Layers 1 and 2 produce TRANSPOSED outputs (lhsT = the small rhs columns,
moving operand = the P tile, N=512): one matmul per (j-tile, layer) instead
of four, one PSUM bank each, and Z is produced directly in [H2, node]
layout for the fp16 Z Z^T decoder (no transposes).

Engine balance per j-tile (tuned from perfetto traces): a custom fused DVE
op RK1MAX (out = max(in0*s0, in1*s1)) builds the score tile in one VectorE
instruction for ~5/9 of tiles; the rest use two ScalarE copy-with-scale ops
plus a VectorE max.  The mask multiply runs as one VectorE op per 8 j-tiles
([128, 8*512] supertile).  GpSimd is deliberately unused for elementwise
work: its ops are ~8x slower and its SBUF-port contention slows concurrent
VectorE ops ~4x.  Short bursts of dummy fp32 matmuls (gated on gathered
data) keep the PE clock-gate (HAM) at 2.4 GHz through the VE-paced loops.
"""

import numpy as np
import ml_dtypes

import concourse.bass as bass
import concourse.mybir as mybir
import concourse.tile as tile
from concourse import bacc
from concourse.bass import ts
from concourse.bass_utils import run_bass_kernel_spmd
from concourse.masks import make_identity

import concourse.dve_ops as _dve_ops
from concourse.dve_spec import Spec as _Spec, Src0 as _Src0, Src1 as _Src1, \
    C0 as _C0, C1 as _C1, maxx as _maxx, lower as _dve_lower
from concourse.dve_uop import DveOpSpec as _DveOpSpec


def _register_rk1max():
    """Custom fused DVE op: out = max(in0*s0, in1*s1) — builds an attention
    P-tile precursor in one VectorE instruction instead of
    (scalar-mul + scalar_tensor_tensor)."""
    name = "RK1MAX"
    if name in _dve_ops._SUB_OPCODE_FOR_NAME:
        return next(o for o in _dve_ops.OPS if o.name == name)
    spec = _Spec(body=_maxx(_Src0 * _C0, _Src1 * _C1))
    row = max(_dve_ops._SUB_OPCODE_FOR_NAME.values()) + 1
    assert row < 0x20
    _dve_ops._SUB_OPCODE_FOR_NAME[name] = row
    shas = {}
    for ver in ("v3", "v4"):
        try:
            r = _DveOpSpec(name=name, opcode=row,
                           uops=_dve_lower(spec, ver=ver), rd1_en=True)
            shas[ver] = r.sha(ver)
        except Exception:
            pass
    op = _dve_ops.DveOp(name, spec, subdim=False, uops_sha=shas)
    _dve_ops.OPS.append(op)
    return op


RK1MAX = _register_rk1max()

F32 = mybir.dt.float32
F32R = mybir.dt.float32r
F16 = mybir.dt.float16
BF16 = mybir.dt.bfloat16
AF = mybir.ActivationFunctionType
OP = mybir.AluOpType

N = 4096
INPUT_DIM = 512
H1 = 256
H2 = 64
ALPHA = 0.2
NCORES = 8
NB = N // NCORES          # 512 rows per core
IT = NB // 128            # 4 i-tiles per core
JT = N // 128             # 32 j-tiles

# G (layer-0 gathered rhs) column layout, width 136:
#   0:64 u1 | 64:128 u2 | 128 p1a | 129 p1b | 130 p2a | 131 p2b
#   | 132 ones | 133 B0 | 134 D0 | 135 pad
GW = 136
# G1 (layers 1+2 gathered rhs) column layout, width 136:
#   0:64 h1 | 64 ones | 65 B1 | 66 D1 | 67:131 h2 | 131 ones
#   | 132 B2 | 133 D2 | 134:136 pad
G1W = 136

def build_program():
    nc = bacc.Bacc("TRN2", target_bir_lowering=False, debug=False,
                   num_devices=NCORES)

    # full X^T (all 4096 nodes), host-rearranged to [128, 4, N]: every core
    # builds the complete layer-0 rhs G locally (redundantly) instead of
    # gathering it — kills the first AllGather and its skew barrier.
    xt = nc.dram_tensor("xt", [128, 4, N], F16, kind="ExternalInput").ap()
    # wbig cols: 0:132 attention rhs (u1|u2|p1a|p1b|p2a|p2b), 132 = f2 weight
    wbig = nc.dram_tensor("wbig", [128, 4, 133], F16,
                          kind="ExternalInput").ap()
    # own rows of X^T + f1 weight column (for the A/C broadcast rows)
    xto = nc.dram_tensor("xto", [128, 4, NB], F16, kind="ExternalInput").ap()
    wfo = nc.dram_tensor("wfo", [128, 4, 1], F16, kind="ExternalInput").ap()
    maskT = nc.dram_tensor("maskT", [N, NB], BF16, kind="ExternalInput").ap()
    noiseT = nc.dram_tensor("noiseT", [H2, NB], F32, kind="ExternalInput").ap()
    # f16 output halves the 8MB/core output DMA; host casts back to f32
    # (sigmoid outputs are in [0,1] -- f16 roundoff ~5e-4 relative)
    apred = nc.dram_tensor("apred", [NB, N], F16, kind="ExternalOutput").ap()

    rg = [list(range(NCORES))]

    with tile.TileContext(nc) as tc, \
         tc.tile_pool(name="perm", bufs=1) as perm, \
         tc.tile_pool(name="gdram", bufs=1, space="DRAM") as gdram:

        # ---------- long-lived tiles ----------
        ident = perm.tile([128, 128], F32)
        make_identity(nc, ident)
        ones1 = perm.tile([1, 128], BF16)
        nc.vector.memset(ones1, 1.0)
        onesr = perm.tile([1, 64], F32R)
        ones64f = perm.tile([1, 64], F32)
        nc.vector.memset(ones64f, 1.0)
        nc.scalar.activation(onesr, ones64f, AF.Copy)

        mask_g = [perm.tile([128, 4, NB], BF16, tag=f"maskg{g}",
                             name=f"maskg{g}") for g in range(JT // 4)]

        bc0a = perm.tile([128, NB], BF16)
        bc0c = perm.tile([128, NB], BF16)
        bc1a = perm.tile([128, NB], BF16)
        bc1c = perm.tile([128, NB], BF16)
        bc2a = perm.tile([128, NB], BF16)
        bc2c = perm.tile([128, NB], BF16)
        bd0 = perm.tile([128, JT, 2], F32)           # f32 B0/D0 scalar cols
        bd12 = perm.tile([128, 2, JT, 2], F32)       # f32 B/D cols layers 1,2
        NQ = JT // 4
        r0q = [perm.tile([128, NQ, GW], BF16, tag=f"r0q{q}", name=f"r0q{q}")
               for q in range(4)]
        r1q = [perm.tile([128, NQ, G1W], BF16, tag=f"r1q{q}", name=f"r1q{q}")
               for q in range(4)]
        noiseT_sb = perm.tile([64, NB], F32)
        nc.sync.dma_start(out=noiseT_sb, in_=noiseT)
        zt_own = perm.tile([64, NB], F16)
        ztb = perm.tile([64, NCORES, NB], F16)

        g1_in = gdram.tile([NB, G1W], BF16)
        g1_out = gdram.tile([N, G1W], BF16, addr_space="Shared")
        # small early gather: per-node (B1, D1, B2, D2) f2'-exp scalars so
        # layers-1/2 score tiles can be built while the big G1 gather flies
        sm_in = gdram.tile([4, NB], F32)
        sm_out = gdram.tile([4 * NCORES, NB], F32, addr_space="Shared")
        ztg_in = gdram.tile([64, NB], F16)
        ztg_out = gdram.tile([NCORES * 64, NB], F16, addr_space="Shared")

        # ------- stage A+C: build FULL G and run layer-0 attention, -------
        # interleaved per quarter.  Every core computes all 4096 rows of the
        # layer-0 attention rhs locally (redundant PE) — no AllGather.  The
        # VE score-tile work for quarter q starts as soon as G-quarter q is
        # built, overlapping the remaining input DMA instead of waiting for
        # the full G.
        with tc.tile_pool(name="bld_sb", bufs=2) as bsb, \
             tc.tile_pool(name="p0_ps", bufs=1, space="PSUM") as p0ps, \
             tc.tile_pool(name="p0_v", bufs=5) as vp:

            NXC = 4                      # X^T DMA chunks (pipelining)
            XW = N // NXC                # 1024 nodes per chunk
            TPC = JT // NXC              # 8 j-tiles per chunk
            wb_sb = bsb.tile([128, 4, 133], F16, tag="wb_sb", bufs=1)
            nc.sync.dma_start(out=wb_sb, in_=wbig)
            wf_sb = bsb.tile([128, 4, 1], F16, tag="wf_sb", bufs=1)
            nc.sync.dma_start(out=wf_sb, in_=wfo)
            xto_sb = bsb.tile([128, 4, NB], F16, tag="xto_sb", bufs=1)
            nc.sync.dma_start(out=xto_sb, in_=xto)
            xt_sb = [bsb.tile([128, 4, XW], F16, tag=f"xt{q}",
                              name=f"xt{q}", bufs=1) for q in range(NXC)]
            # priority-interleaved input DMAs: xt chunk q right before the
            # two mask groups consumed with it, so quarter-q compute never
            # waits on bytes it doesn't need yet
            for q in range(NXC):
                nc.sync.dma_start(out=xt_sb[q],
                                  in_=xt[:, :, q * XW:(q + 1) * XW])
                for g in (2 * q, 2 * q + 1):
                    nc.sync.dma_start(
                        out=mask_g[g],
                        in_=maskT[g * 512:(g + 1) * 512, :]
                        .rearrange("(t p) i -> p t i", p=128))

            a0row = bsb.tile([1, NB], BF16, tag="a0row", bufs=1)
            c0row = bsb.tile([1, NB], BF16, tag="c0row", bufs=1)

            # own f1 -> exp'd A0/C0 rows, then broadcast tiles
            with tc.tile_pool(name="bld_ps0", bufs=1,
                              space="PSUM") as bps0:
                for s in range(IT):
                    psF = bps0.tile([128, 1], F32, tag="psF")
                    for k in range(4):
                        nc.tensor.matmul(psF, lhsT=xto_sb[:, k, ts(s, 128)],
                                         rhs=wf_sb[:, k, :],
                                         start=(k == 0), stop=(k == 3))
                    fcol = bsb.tile([128, 1], F32, tag="fcol")
                    nc.scalar.activation(fcol, psF, AF.Copy)
                    psT = bps0.tile([1, 128], F32, tag="psT")
                    nc.tensor.transpose(psT, fcol, ident)
                    nc.scalar.activation(a0row[0:1, ts(s, 128)], psT, AF.Exp)
                    nc.scalar.activation(c0row[0:1, ts(s, 128)], psT, AF.Exp,
                                         scale=ALPHA)
                for dst, row in ((bc0a, a0row), (bc0c, c0row)):
                    psB = bps0.tile([128, NB], F32, tag="psB")
                    nc.tensor.matmul(psB, lhsT=ones1, rhs=row, start=True,
                                     stop=True)
                    nc.scalar.activation(dst, psB, AF.Copy)

            ps0 = [p0ps.tile([128, 133], F32, tag=f"ps0_{s}",
                             name=f"ps0_{s}") for s in range(IT)]
            psW = p0ps.tile([128, 128], F32, tag="psW")

            with tc.tile_pool(name="bld_ps", bufs=2, space="PSUM") as bps:
                for q in range(4):
                    # G rows for quarter q, straight into r0q layout;
                    # B0/D0 exps go directly to the f32 bd0 scalar tile
                    for t in range(q * NQ, (q + 1) * NQ):
                        r = t % NQ
                        psA = bps.tile([128, 133], F32, tag="psA")
                        xs = xt_sb[t // TPC]
                        for k in range(4):
                            nc.tensor.matmul(
                                psA, lhsT=xs[:, k, ts(t % TPC, 128)],
                                rhs=wb_sb[:, k, :], start=(k == 0),
                                stop=(k == 3))
                        nc.vector.tensor_copy(r0q[q][:, r, 0:132],
                                              psA[:, 0:132])
                        nc.vector.memset(r0q[q][:, r, 132:133], 1.0)
                        nc.scalar.activation(bd0[:, t, 0:1],
                                             psA[:, 132:133], AF.Exp)
                        nc.scalar.activation(bd0[:, t, 1:2],
                                             psA[:, 132:133], AF.Exp,
                                             scale=ALPHA)
                    if q == 0:
                        # HAM warmup: fp32 matmul burst right before the
                        # first real attention matmuls flips the PE clock
                        # gate to 2.4 GHz
                        nc.tensor.matmul(
                            psW[:, 0:16], lhsT=ident,
                            rhs=bd0[:, 0:NQ, :]
                            .rearrange("p t c -> p (t c)"),
                            start=True, stop=True)
                        for w in range(10):
                            nc.tensor.matmul(psW, lhsT=ident, rhs=ident,
                                             start=True, stop=True)
                    for g in (2 * q, 2 * q + 1):
                        t3s = vp.tile([128, 4, NB], BF16, tag="t3s",
                                      name=f"t3s0_{g}")
                        for u in range(4):
                            t = 4 * g + u
                            if t % 9 < 4:
                                t1 = vp.tile([128, NB], BF16, tag="t1")
                                nc.scalar.activation(t1, bc0a, AF.Copy,
                                                     scale=bd0[:, t, 0:1])
                                t2 = vp.tile([128, NB], BF16, tag="t2")
                                nc.scalar.activation(t2, bc0c, AF.Copy,
                                                     scale=bd0[:, t, 1:2])
                                nc.vector.tensor_tensor(t3s[:, u, :], t1, t2,
                                                        op=OP.max)
                            else:
                                nc.vector._custom_dve(
                                    RK1MAX, out=t3s[:, u, :], in0=bc0a,
                                    in1=bc0c, s0=bd0[:, t, 0:1],
                                    s1=bd0[:, t, 1:2])
                        pts = vp.tile([128, 4, NB], BF16, tag="pts",
                                      name=f"pts0_{g}")
                        nc.vector.tensor_tensor(pts, t3s, mask_g[g],
                                                op=OP.mult)
                        # keep the PE clock gate warm through the VE loop
                        nc.tensor.matmul(psW, lhsT=ident, rhs=ident,
                                         start=True, stop=True)
                        nc.tensor.matmul(psW, lhsT=ident, rhs=ident,
                                         start=True, stop=True)
                        for u in range(4):
                            t = 4 * g + u
                            for s in range(IT):
                                nc.tensor.matmul(
                                    ps0[s], lhsT=pts[:, u, ts(s, 128)],
                                    rhs=r0q[t // NQ][:, t % NQ, 0:133],
                                    start=(t == 0), stop=(t == JT - 1))

            # ---------------- stage D: normalize + build G1 ----------------
            with tc.tile_pool(name="d_sb", bufs=2) as dsb, \
                 tc.tile_pool(name="d_ps", bufs=1, space="PSUM") as dps:

                rows12 = dsb.tile([1, 4, NB], BF16, tag="rows12", bufs=1)

                # one consolidated reciprocal for all 4 i-tiles (VE recip has
                # a ~1.2us fixed cost -- pay it once, not four times)
                den4 = dsb.tile([128, 4], F32, tag="den4", bufs=1)
                for s in range(IT):
                    nc.vector.tensor_copy(den4[:, s:s + 1],
                                          ps0[s][:, 132:133])
                r0c4 = dsb.tile([128, 4], F32, tag="r0c4", bufs=1)
                nc.vector.reciprocal(r0c4, den4)
                r0a4 = dsb.tile([128, 4], F32, tag="r0a4", bufs=1)
                nc.vector.tensor_scalar_mul(r0a4, r0c4, ALPHA)

                # f2' exps -> smallc columns [B1|D1|B2|D2], transpose to the
                # [4, NB] gather payload, and launch the SMALL AllGather
                # before the g1own builds so it absorbs the entry skew
                smallT = dsb.tile([4, NB], F32, tag="smallT", bufs=1)
                smallcs = []
                for s in range(IT):
                    r0c = r0c4[:, s:s + 1]
                    r0a = r0a4[:, s:s + 1]
                    smallc = dsb.tile([128, 4], F32, tag="smallc",
                                      name=f"smallc{s}", bufs=4)
                    nc.scalar.activation(smallc[:, 0:1], ps0[s][:, 129:130],
                                         AF.Exp, scale=r0c)
                    nc.scalar.activation(smallc[:, 1:2], ps0[s][:, 129:130],
                                         AF.Exp, scale=r0a)
                    nc.scalar.activation(smallc[:, 2:3], ps0[s][:, 131:132],
                                         AF.Exp, scale=r0c)
                    nc.scalar.activation(smallc[:, 3:4], ps0[s][:, 131:132],
                                         AF.Exp, scale=r0a)
                    smallcs.append(smallc)
                    psSm = dps.tile([4, 128], F32, tag="psT2")
                    nc.tensor.transpose(psSm, smallc, ident)
                    nc.vector.tensor_copy(smallT[:, ts(s, 128)], psSm)
                nc.sync.dma_start(out=sm_in, in_=smallT)
                nc.gpsimd.collective_compute(
                    "AllGather", OP.bypass, replica_groups=rg,
                    ins=[sm_in.opt()], outs=[sm_out.opt()])

                for s in range(IT):
                    r0c = r0c4[:, s:s + 1]
                    g1own = dsb.tile([128, G1W], BF16, tag="g1own")
                    nc.vector.tensor_scalar_mul(g1own[:, 0:64],
                                                ps0[s][:, 0:64], r0c)
                    nc.vector.memset(g1own[:, 64:65], 1.0)
                    nc.vector.tensor_copy(g1own[:, 65:67],
                                          smallcs[s][:, 0:2])
                    nc.vector.tensor_scalar_mul(g1own[:, 67:131],
                                                ps0[s][:, 64:128], r0c)
                    nc.vector.memset(g1own[:, 131:132], 1.0)
                    nc.vector.tensor_copy(g1own[:, 132:134],
                                          smallcs[s][:, 2:4])
                    nc.vector.memset(g1own[:, 134:136], 0.0)
                    nc.sync.dma_start(out=g1_in[ts(s, 128), :], in_=g1own)

                # small-gather payload back in; ALSO serializes the two
                # AllGathers: the gpsimd dep copy below keeps the big-AG
                # trigger behind the small AG's completion.  Two concurrent
                # in-flight collectives can cross-block on the CC cores
                # (one rank's ncfw picks up the big one first and blocks,
                # never serving the small one another rank waits on).
                sm_sb = dsb.tile([32, NB], F32, tag="sm_sb", bufs=1)
                nc.sync.dma_start(out=sm_sb, in_=sm_out)
                agdep = dsb.tile([1, 1], F32, tag="agdep", bufs=1)
                nc.gpsimd.tensor_copy(agdep, sm_sb[0:1, 0:1])
                nc.gpsimd.collective_compute(
                    "AllGather", OP.bypass, replica_groups=rg,
                    ins=[g1_in.opt()], outs=[g1_out.opt()])

                # f1' (col 128) and f1'' (col 130) -> exp'd rows; runs on
                # ACT/PE while the gathers are in flight
                for s in range(IT):
                    r0c = r0c4[:, s:s + 1]
                    for li, col in ((0, 128), (2, 130)):
                        fcl = dsb.tile([128, 1], F32, tag="fcl")
                        nc.scalar.activation(fcl, ps0[s][:, col:col + 1],
                                             AF.Copy, scale=r0c)
                        psT2 = dps.tile([1, 128], F32, tag="psT2")
                        nc.tensor.transpose(psT2, fcl, ident)
                        nc.scalar.activation(rows12[0:1, li, ts(s, 128)],
                                             psT2, AF.Exp)
                        nc.scalar.activation(rows12[0:1, li + 1, ts(s, 128)],
                                             psT2, AF.Exp, scale=ALPHA)

                for i, dst in enumerate((bc1a, bc1c, bc2a, bc2c)):
                    psB2 = dps.tile([128, NB], F32, tag="psB2")
                    nc.tensor.matmul(psB2, lhsT=ones1,
                                     rhs=rows12[0:1, i, :], start=True,
                                     stop=True)
                    nc.scalar.activation(dst, psB2, AF.Copy)

                # transpose the small-gather payload into bd12 layout
                for tr in range(4):
                    psBD = dps.tile([128, 32], F32, tag="psB2",
                                    name=f"psBD{tr}")
                    nc.tensor.transpose(psBD, sm_sb[:, ts(tr, 128)],
                                        ident[0:32, 0:32])
                    src = psBD.rearrange("p (r g) -> p r g", g=4)
                    for l in range(2):
                        dst = bd12[:, l, :, :].rearrange(
                            "p (r t4) c -> p t4 r c", t4=4)[:, tr]
                        nc.vector.tensor_copy(dst, src[:, :, 2 * l:2 * l + 2])

                for q in range(4):
                    nc.sync.dma_start(
                        out=r1q[q],
                        in_=g1_out[q * NQ * 128:(q + 1) * NQ * 128, :]
                        .rearrange("(t p) c -> p t c", p=128))

        # -------- stage E: layers 1+2, interleaved, transposed outputs -----
        # psT[c, i] = sum_j G1[j, c] * P[j, i]; row 64 = denominator.
        with tc.tile_pool(name="e_ps", bufs=1, space="PSUM") as eps, \
             tc.tile_pool(name="e_v", bufs=5) as vpl, \
             tc.tile_pool(name="e_sb", bufs=1) as esb:

            ps1T = eps.tile([65, NB], F32, tag="ps1T")
            ps2T = eps.tile([65, NB], F32, tag="ps2T")
            psW2 = eps.tile([128, 128], F32, tag="psW2")
            # HAM warm burst gated on the gathered G1 (the matmuls' actual
            # dependency) -- fires right as the big AllGather lands, while
            # the VE is still draining prebuilt score tiles
            nc.tensor.matmul(psW2, lhsT=r1q[0][:, 0, 0:128],
                             rhs=bc1a[:, 0:128], start=True, stop=True)
            for w in range(16):
                nc.tensor.matmul(psW2, lhsT=ident, rhs=ident,
                                 start=True, stop=True)

            def p_group(g, uniq, bca, bcc, bd, pool):
                # deep pts/t3s rings: score tiles only need bd12 (small
                # gather) + local bcasts + mask, so the VE prebuilds them
                # while the big G1 AllGather is still in flight
                t3s = pool.tile([128, 4, NB], BF16, tag="t3s", bufs=4,
                                name=f"t3se_{uniq}_{g}")
                for u in range(4):
                    t = 4 * g + u
                    if (t + 2 * uniq) % 15 < 8:
                        t1 = pool.tile([128, NB], BF16, tag="t1", bufs=6,
                                       name=f"t1e_{uniq}_{t}")
                        nc.scalar.activation(t1, bca, AF.Copy,
                                             scale=bd[:, t, 0:1])
                        t2 = pool.tile([128, NB], BF16, tag="t2", bufs=6,
                                       name=f"t2e_{uniq}_{t}")
                        nc.scalar.activation(t2, bcc, AF.Copy,
                                             scale=bd[:, t, 1:2])
                        nc.vector.tensor_tensor(t3s[:, u, :], t1, t2,
                                                op=OP.max)
                    else:
                        nc.vector._custom_dve(
                            RK1MAX, out=t3s[:, u, :], in0=bca, in1=bcc,
                            s0=bd[:, t, 0:1], s1=bd[:, t, 1:2])
                pts = pool.tile([128, 4, NB], BF16, tag="pts", bufs=16,
                                name=f"ptse_{uniq}_{g}")
                nc.vector.tensor_tensor(pts, t3s, mask_g[g], op=OP.mult)
                # keep the PE clock-gate warm through the VE-paced drain
                nc.tensor.matmul(psW2, lhsT=ident, rhs=ident,
                                 start=True, stop=True)
                nc.tensor.matmul(psW2, lhsT=ident, rhs=ident,
                                 start=True, stop=True)
                return pts

            # pass 2 (logstd) first so its Z-chain overlaps pass 1
            for g in range(JT // 4):
                pts = p_group(g, 2, bc2a, bc2c, bd12[:, 1, :, :], vpl)
                for u in range(4):
                    t = 4 * g + u
                    nc.tensor.matmul(ps2T,
                                     lhsT=r1q[t // NQ][:, t % NQ, 67:132],
                                     rhs=pts[:, u, :],
                                     start=(t == 0), stop=(t == JT - 1))

            r2row = esb.tile([1, NB], F32)
            nc.vector.reciprocal(r2row, ps2T[64:65, :])
            r2r = esb.tile([1, NB], F32R)
            nc.scalar.activation(r2r, r2row, AF.Copy)
            psBC2 = eps.tile([64, NB], F32, tag="psBC2")
            nc.tensor.matmul(psBC2, lhsT=onesr, rhs=r2r, start=True,
                             stop=True)
            r2bc = esb.tile([64, NB], F32)
            nc.scalar.activation(r2bc, psBC2, AF.Copy)
            ltT = esb.tile([64, NB], F32)
            nc.vector.tensor_tensor(ltT, ps2T[0:64, :], r2bc, op=OP.mult)
            eT = esb.tile([64, NB], F32)
            nc.scalar.activation(eT, ltT, AF.Exp)
            zmT = esb.tile([64, NB], F32)
            nc.vector.tensor_tensor(zmT, eT, noiseT_sb, op=OP.mult)

            for g in range(JT // 4):
                pts = p_group(g, 1, bc1a, bc1c, bd12[:, 0, :, :], vpl)
                for u in range(4):
                    t = 4 * g + u
                    nc.tensor.matmul(ps1T,
                                     lhsT=r1q[t // NQ][:, t % NQ, 0:65],
                                     rhs=pts[:, u, :],
                                     start=(t == 0), stop=(t == JT - 1))

            r1row = esb.tile([1, NB], F32)
            nc.vector.reciprocal(r1row, ps1T[64:65, :])
            r1r = esb.tile([1, NB], F32R)
            nc.scalar.activation(r1r, r1row, AF.Copy)
            psBC1 = eps.tile([64, NB], F32, tag="psBC1")
            nc.tensor.matmul(psBC1, lhsT=onesr, rhs=r1r, start=True,
                             stop=True)
            r1bc = esb.tile([64, NB], F32)
            nc.scalar.activation(r1bc, psBC1, AF.Copy)
            meanT = esb.tile([64, NB], F32)
            nc.vector.tensor_tensor(meanT, ps1T[0:64, :], r1bc, op=OP.mult)
            zT = esb.tile([64, NB], F32)
            nc.vector.tensor_tensor(zT, zmT, meanT, op=OP.add)
            nc.scalar.activation(zt_own, zT, AF.Copy)

        # ---------------- stage F: gather Z^T -----------------------------
        nc.sync.dma_start(out=ztg_in, in_=zt_own)
        nc.gpsimd.collective_compute(
            "AllGather", OP.bypass, replica_groups=rg,
            ins=[ztg_in.opt()], outs=[ztg_out.opt()])
        nc.sync.dma_start(
            out=ztb, in_=ztg_out.rearrange("(b p) i -> p b i", p=64))

        # ---------------- stage G: decoder sigmoid(Z @ Z^T) ----------------
        with tc.tile_pool(name="dec_ps", bufs=3, space="PSUM") as decps, \
             tc.tile_pool(name="dec_sb", bufs=3) as decsb:
            # own (diagonal) blocks first — pure warm-up compute that
            # overlaps the Z^T gather (its output position would be
            # core-dependent, so the result is discarded; the paired loop
            # recomputes it)
            for s in range(IT):
                psD = decps.tile([128, NB], F32, tag="psDd",
                                 name=f"psDd_{s}", bufs=1)
                nc.tensor.matmul(psD, lhsT=zt_own[:, ts(s, 128)],
                                 rhs=zt_own, start=True, stop=True)
                osb = decsb.tile([128, NB], F16, tag="osbd",
                                 name=f"osbd_{s}", bufs=1)
                nc.scalar.activation(osb, psD, AF.Sigmoid)

            # PE warm burst during the rest of the Z^T gather window: one
            # zt-gated matmul, then fp32 ident matmuls -- only fp32 PE work
            # reliably flips the HAM clock gate to 2.4 GHz (the f16 decoder
            # stream alone never does; see stage-C/E bursts)
            psWd = decps.tile([128, NB], F32, tag="psWd", bufs=1)
            nc.tensor.matmul(psWd, lhsT=zt_own[:, 0:128], rhs=zt_own,
                             start=True, stop=True)
            for w in range(22):
                nc.tensor.matmul(psWd[:, 0:128], lhsT=ident, rhs=ident,
                                 start=True, stop=True)
            # short re-warm burst gated on the gathered Z^T
            for w in range(4):
                nc.tensor.matmul(psWd, lhsT=ztb[:, 0, 0:128],
                                 rhs=ztb[:, w % 2, :], start=True, stop=True)

            # paired j-blocks: 2 matmuls into one 2-bank PSUM tile, then a
            # single [128, 1024] sigmoid and a single contiguous DMA out
            for s in range(IT):
                for bp in range(NCORES // 2):
                    psD2 = decps.tile([128, 2, NB], F32, tag="psD2",
                                      name=f"psD2_{s}_{bp}", bufs=3)
                    nc.tensor.matmul(psD2[:, 0, :],
                                     lhsT=zt_own[:, ts(s, 128)],
                                     rhs=ztb[:, 2 * bp, :],
                                     start=True, stop=True)
                    nc.tensor.matmul(psD2[:, 1, :],
                                     lhsT=zt_own[:, ts(s, 128)],
                                     rhs=ztb[:, 2 * bp + 1, :],
                                     start=True, stop=True)
                    osb2 = decsb.tile([128, 2, NB], F16, tag="osb2",
                                      name=f"osb2_{s}_{bp}", bufs=6)
                    nc.scalar.activation(osb2, psD2, AF.Sigmoid)
                    eng = nc.sync if bp % 2 == 0 else nc.gpsimd
                    eng.dma_start(
                        out=apred[ts(s, 128), ts(bp, 2 * NB)], in_=osb2)
                    # fp32 ident matmul per pair keeps the clock gate hot
                    # through the sigmoid/DMA-paced drain
                    nc.tensor.matmul(psWd[:, 0:128], lhsT=ident, rhs=ident,
                                     start=True, stop=True)

    nc.compile()
    return nc


_program = None


def _get_program():
    global _program
    if _program is None:
        _program = build_program()
    return _program


def kernel(X, adj, noise, W0, a0, W1, a1, W2, a2, _trace=False):
    X = np.asarray(X, dtype=np.float32)
    adj = np.asarray(adj)
    noise = np.asarray(noise, dtype=np.float32)
    W0 = np.asarray(W0, dtype=np.float32)
    a0 = np.asarray(a0, dtype=np.float32)
    W1 = np.asarray(W1, dtype=np.float32)
    a1 = np.asarray(a1, dtype=np.float32)
    W2 = np.asarray(W2, dtype=np.float32)
    a2 = np.asarray(a2, dtype=np.float32)

    # folded weight matrix [512, 133]: attention rhs cols + f2 weight
    u1 = W0 @ W1
    u2 = W0 @ W2
    wbig = np.concatenate([
        u1, u2,
        u1 @ a1[:H2], u1 @ a1[H2:],
        u2 @ a2[:H2], u2 @ a2[H2:],
        W0 @ a0[H1:],
    ], axis=1).astype(np.float32)
    wfo = (W0 @ a0[:H1]).astype(np.float32)  # [512, 1] f1 weight

    maskT = adj.astype(ml_dtypes.bfloat16).T  # 0/1, exact in bf16

    def rearr(m):
        # [512, c] -> [128, 4, c] matching the device-side k-split
        c = m.shape[1]
        return np.ascontiguousarray(
            m.reshape(4, 128, c).transpose(1, 0, 2)).astype(np.float16)

    xt_full = rearr(np.ascontiguousarray(X.T))     # [128, 4, 4096], shared
    wbig_r = rearr(wbig)
    wfo_r = rearr(wfo)

    in_maps = []
    for k in range(NCORES):
        sl = slice(k * NB, (k + 1) * NB)
        in_maps.append({
            "xt": xt_full,
            "wbig": wbig_r,
            "xto": rearr(np.ascontiguousarray(X[sl].T)),
            "wfo": wfo_r,
            "maskT": np.ascontiguousarray(maskT[:, sl]),
            "noiseT": np.ascontiguousarray(noise[sl].T),
        })

    nc = _get_program()
    res = run_bass_kernel_spmd(nc, in_maps, core_ids=list(range(NCORES)),
                               trace=_trace)
    out = np.concatenate(
        [res.results[k]["apred"].astype(np.float32) for k in range(NCORES)],
        axis=0)
    if _trace:
        kernel.last_results = res
    return out



# revision 26
# speedup vs baseline: 1.1386x; 1.1386x over previous
"""Trainium2 Bass kernel for nn_AVGAE (3-layer GAT variational graph
autoencoder, N=4096) on 8 NeuronCores.

Sharding: 1D row partition of the N x N attention/score matrices — core k
owns output rows [512k, 512k+512). Small per-node features are all-gathered
between layers (AllGather over internal DRAM tiles).

Key algebraic restructuring (no elementwise transcendentals over N x N):
  exp(leaky_relu(f1_i + f2_j, a)) = max(A_i*B_j, C_i*D_j)
  with A=exp(f1), B=exp(f2), C=exp(a*f1), D=exp(a*f2)
so each N x N score tile is built with vector ALU ops only (outer-product
scalar muls + max + mask mul), all bf16, then consumed directly by the
tensor engine as attention weights.  Softmax denominators come for free as
a ones-column in the attention rhs (exp(MASK_VAL) == 0 exactly in fp32, so
masked entries contribute 0 to numerator and denominator, matching the
reference softmax).

All per-node "h" quantities of layers 1/2 are linear images of layer-0
attention output, so the layer-0 attention rhs carries
[h0@W1 | h0@W2 | per-layer score vectors | ones] and hidden itself is never
materialized.  Host precomputes the folded weight matrix.  Every core
builds the FULL 4096-row layer-0 rhs G locally from the full X (f16) —
~15us of redundant PE beats the ~25us fixed latency + skew barrier of an
AllGather (collectives here cost ~25us regardless of payload size, so the
kernel keeps exactly two: the g1 feature gather and the Z^T gather).

Layers 1 and 2 produce TRANSPOSED outputs (lhsT = the small rhs columns,
moving operand = the P tile, N=512): one matmul per (j-tile, layer) instead
of four, one PSUM bank each, and Z is produced directly in [H2, node]
layout for the fp16 Z Z^T decoder (no transposes).

Engine balance per j-tile (tuned from perfetto traces): a custom fused DVE
op RK1MAX (out = max(in0*s0, in1*s1)) builds the score tile in one VectorE
instruction for ~5/9 of tiles; the rest use two ScalarE copy-with-scale ops
plus a VectorE max.  The mask multiply runs as one VectorE op per 8 j-tiles
([128, 8*512] supertile).  GpSimd is deliberately unused for elementwise
work: its ops are ~8x slower and its SBUF-port contention slows concurrent
VectorE ops ~4x.  Short bursts of dummy fp32 matmuls (gated on gathered
data) keep the PE clock-gate (HAM) at 2.4 GHz through the VE-paced loops.
"""

import numpy as np
import ml_dtypes

import concourse.bass as bass
import concourse.mybir as mybir
import concourse.tile as tile
from concourse import bacc
from concourse.bass import ts
from concourse.bass_utils import run_bass_kernel_spmd
from concourse.masks import make_identity

import concourse.dve_ops as _dve_ops
from concourse.dve_spec import Spec as _Spec, Src0 as _Src0, Src1 as _Src1, \
    C0 as _C0, C1 as _C1, maxx as _maxx, lower as _dve_lower
from concourse.dve_uop import DveOpSpec as _DveOpSpec


def _register_rk1max():
    """Custom fused DVE op: out = max(in0*s0, in1*s1) — builds an attention
    P-tile precursor in one VectorE instruction instead of
    (scalar-mul + scalar_tensor_tensor)."""
    name = "RK1MAX"
    if name in _dve_ops._SUB_OPCODE_FOR_NAME:
        return next(o for o in _dve_ops.OPS if o.name == name)
    spec = _Spec(body=_maxx(_Src0 * _C0, _Src1 * _C1))
    row = max(_dve_ops._SUB_OPCODE_FOR_NAME.values()) + 1
    assert row < 0x20
    _dve_ops._SUB_OPCODE_FOR_NAME[name] = row
    shas = {}
    for ver in ("v3", "v4"):
        try:
            r = _DveOpSpec(name=name, opcode=row,
                           uops=_dve_lower(spec, ver=ver), rd1_en=True)
            shas[ver] = r.sha(ver)
        except Exception:
            pass
    op = _dve_ops.DveOp(name, spec, subdim=False, uops_sha=shas)
    _dve_ops.OPS.append(op)
    return op


RK1MAX = _register_rk1max()

F32 = mybir.dt.float32
F32R = mybir.dt.float32r
F16 = mybir.dt.float16
BF16 = mybir.dt.bfloat16
AF = mybir.ActivationFunctionType
OP = mybir.AluOpType

N = 4096
INPUT_DIM = 512
H1 = 256
H2 = 64
ALPHA = 0.2
NCORES = 8
NB = N // NCORES          # 512 rows per core
IT = NB // 128            # 4 i-tiles per core
JT = N // 128             # 32 j-tiles

# G (layer-0 gathered rhs) column layout, width 136:
#   0:64 u1 | 64:128 u2 | 128 p1a | 129 p1b | 130 p2a | 131 p2b
#   | 132 ones | 133 B0 | 134 D0 | 135 pad
GW = 136
# G1 (layers 1+2 gathered rhs) column layout, width 136:
#   0:64 h1 | 64 ones | 65 B1 | 66 D1 | 67:131 h2 | 131 ones
#   | 132 B2 | 133 D2 | 134:136 pad
G1W = 136

def build_program():
    nc = bacc.Bacc("TRN2", target_bir_lowering=False, debug=False,
                   num_devices=NCORES)

    # full X^T (all 4096 nodes), host-rearranged to [128, 4, N]: every core
    # builds the complete layer-0 rhs G locally (redundantly) instead of
    # gathering it — kills the first AllGather and its skew barrier.
    xt = nc.dram_tensor("xt", [128, 4, N], F16, kind="ExternalInput").ap()
    # wbig cols: 0:132 attention rhs (u1|u2|p1a|p1b|p2a|p2b), 132 = f2 weight
    wbig = nc.dram_tensor("wbig", [128, 4, 133], F16,
                          kind="ExternalInput").ap()
    # own rows of X^T + f1 weight column (for the A/C broadcast rows)
    xto = nc.dram_tensor("xto", [128, 4, NB], F16, kind="ExternalInput").ap()
    wfo = nc.dram_tensor("wfo", [128, 4, 1], F16, kind="ExternalInput").ap()
    maskT = nc.dram_tensor("maskT", [N, NB], BF16, kind="ExternalInput").ap()
    noiseT = nc.dram_tensor("noiseT", [H2, NB], F32, kind="ExternalInput").ap()
    # f16 output halves the 8MB/core output DMA; host casts back to f32
    # (sigmoid outputs are in [0,1] -- f16 roundoff ~5e-4 relative)
    apred = nc.dram_tensor("apred", [NB, N], F16, kind="ExternalOutput").ap()

    rg = [list(range(NCORES))]

    with tile.TileContext(nc) as tc, \
         tc.tile_pool(name="perm", bufs=1) as perm, \
         tc.tile_pool(name="gdram", bufs=1, space="DRAM") as gdram:

        # ---------- long-lived tiles ----------
        ident = perm.tile([128, 128], F32)
        make_identity(nc, ident)
        ones1 = perm.tile([1, 128], BF16)
        nc.vector.memset(ones1, 1.0)
        onesr = perm.tile([1, 64], F32R)
        ones64f = perm.tile([1, 64], F32)
        nc.vector.memset(ones64f, 1.0)
        nc.scalar.activation(onesr, ones64f, AF.Copy)

        mask_g = [perm.tile([128, 4, NB], BF16, tag=f"maskg{g}",
                             name=f"maskg{g}") for g in range(JT // 4)]

        bc0a = perm.tile([128, NB], BF16)
        bc0c = perm.tile([128, NB], BF16)
        bc1a = perm.tile([128, NB], BF16)
        bc1c = perm.tile([128, NB], BF16)
        bc2a = perm.tile([128, NB], BF16)
        bc2c = perm.tile([128, NB], BF16)
        bd0 = perm.tile([128, JT, 2], F32)           # f32 B0/D0 scalar cols
        bd12 = perm.tile([128, 2, JT, 2], F32)       # f32 B/D cols layers 1,2
        NQ = JT // 4
        r0q = [perm.tile([128, NQ, GW], BF16, tag=f"r0q{q}", name=f"r0q{q}")
               for q in range(4)]
        r1q = [perm.tile([128, NQ, G1W], BF16, tag=f"r1q{q}", name=f"r1q{q}")
               for q in range(4)]
        noiseT_sb = perm.tile([64, NB], F32)
        nc.sync.dma_start(out=noiseT_sb, in_=noiseT)
        zt_own = perm.tile([64, NB], F16)
        ztb = perm.tile([64, NCORES, NB], F16)

        g1_in = gdram.tile([NB, G1W], BF16)
        g1_out = gdram.tile([N, G1W], BF16, addr_space="Shared")
        # small early gather: per-node (B1, D1, B2, D2) f2'-exp scalars so
        # layers-1/2 score tiles can be built while the big G1 gather flies
        sm_in = gdram.tile([4, NB], F32)
        sm_out = gdram.tile([4 * NCORES, NB], F32, addr_space="Shared")
        ztg_in = gdram.tile([64, NB], F16)
        ztg_out = gdram.tile([NCORES * 64, NB], F16, addr_space="Shared")

        # ------- stage A+C: build FULL G and run layer-0 attention, -------
        # interleaved per quarter.  Every core computes all 4096 rows of the
        # layer-0 attention rhs locally (redundant PE) — no AllGather.  The
        # VE score-tile work for quarter q starts as soon as G-quarter q is
        # built, overlapping the remaining input DMA instead of waiting for
        # the full G.
        with tc.tile_pool(name="bld_sb", bufs=2) as bsb, \
             tc.tile_pool(name="p0_ps", bufs=1, space="PSUM") as p0ps, \
             tc.tile_pool(name="p0_v", bufs=5) as vp:

            NXC = 4                      # X^T DMA chunks (pipelining)
            XW = N // NXC                # 1024 nodes per chunk
            TPC = JT // NXC              # 8 j-tiles per chunk
            wb_sb = bsb.tile([128, 4, 133], F16, tag="wb_sb", bufs=1)
            nc.sync.dma_start(out=wb_sb, in_=wbig)
            wf_sb = bsb.tile([128, 4, 1], F16, tag="wf_sb", bufs=1)
            nc.sync.dma_start(out=wf_sb, in_=wfo)
            xto_sb = bsb.tile([128, 4, NB], F16, tag="xto_sb", bufs=1)
            nc.sync.dma_start(out=xto_sb, in_=xto)
            xt_sb = [bsb.tile([128, 4, XW], F16, tag=f"xt{q}",
                              name=f"xt{q}", bufs=1) for q in range(NXC)]
            # priority-interleaved input DMAs: xt chunk q right before the
            # two mask groups consumed with it, so quarter-q compute never
            # waits on bytes it doesn't need yet
            for q in range(NXC):
                nc.sync.dma_start(out=xt_sb[q],
                                  in_=xt[:, :, q * XW:(q + 1) * XW])
                for g in (2 * q, 2 * q + 1):
                    nc.sync.dma_start(
                        out=mask_g[g],
                        in_=maskT[g * 512:(g + 1) * 512, :]
                        .rearrange("(t p) i -> p t i", p=128))

            a0row = bsb.tile([1, NB], BF16, tag="a0row", bufs=1)
            c0row = bsb.tile([1, NB], BF16, tag="c0row", bufs=1)

            # own f1 -> exp'd A0/C0 rows, then broadcast tiles
            with tc.tile_pool(name="bld_ps0", bufs=1,
                              space="PSUM") as bps0:
                for s in range(IT):
                    psF = bps0.tile([128, 1], F32, tag="psF")
                    for k in range(4):
                        nc.tensor.matmul(psF, lhsT=xto_sb[:, k, ts(s, 128)],
                                         rhs=wf_sb[:, k, :],
                                         start=(k == 0), stop=(k == 3))
                    fcol = bsb.tile([128, 1], F32, tag="fcol")
                    nc.scalar.activation(fcol, psF, AF.Copy)
                    psT = bps0.tile([1, 128], F32, tag="psT")
                    nc.tensor.transpose(psT, fcol, ident)
                    nc.scalar.activation(a0row[0:1, ts(s, 128)], psT, AF.Exp)
                    nc.scalar.activation(c0row[0:1, ts(s, 128)], psT, AF.Exp,
                                         scale=ALPHA)
                for dst, row in ((bc0a, a0row), (bc0c, c0row)):
                    psB = bps0.tile([128, NB], F32, tag="psB")
                    nc.tensor.matmul(psB, lhsT=ones1, rhs=row, start=True,
                                     stop=True)
                    nc.scalar.activation(dst, psB, AF.Copy)

            ps0 = [p0ps.tile([128, 133], F32, tag=f"ps0_{s}",
                             name=f"ps0_{s}") for s in range(IT)]
            psW = p0ps.tile([128, 128], F32, tag="psW")

            with tc.tile_pool(name="bld_ps", bufs=2, space="PSUM") as bps:

                def build_g_quarter(q):
                    # G rows for quarter q, straight into r0q layout;
                    # B0/D0 exps go directly to the f32 bd0 scalar tile
                    for t in range(q * NQ, (q + 1) * NQ):
                        r = t % NQ
                        psA = bps.tile([128, 133], F32, tag="psA",
                                       name=f"psA_{t}")
                        xs = xt_sb[t // TPC]
                        for k in range(4):
                            nc.tensor.matmul(
                                psA, lhsT=xs[:, k, ts(t % TPC, 128)],
                                rhs=wb_sb[:, k, :], start=(k == 0),
                                stop=(k == 3))
                        nc.vector.tensor_copy(r0q[q][:, r, 0:132],
                                              psA[:, 0:132])
                        nc.vector.memset(r0q[q][:, r, 132:133], 1.0)
                        nc.scalar.activation(bd0[:, t, 0:1],
                                             psA[:, 132:133], AF.Exp)
                        nc.scalar.activation(bd0[:, t, 1:2],
                                             psA[:, 132:133], AF.Exp,
                                             scale=ALPHA)

                def score_group0(g):
                    t3s = vp.tile([128, 4, NB], BF16, tag="t3s",
                                  name=f"t3s0_{g}")
                    for u in range(4):
                        t = 4 * g + u
                        if t % 9 < 4:
                            t1 = vp.tile([128, NB], BF16, tag="t1")
                            nc.scalar.activation(t1, bc0a, AF.Copy,
                                                 scale=bd0[:, t, 0:1])
                            t2 = vp.tile([128, NB], BF16, tag="t2")
                            nc.scalar.activation(t2, bc0c, AF.Copy,
                                                 scale=bd0[:, t, 1:2])
                            nc.vector.tensor_tensor(t3s[:, u, :], t1, t2,
                                                    op=OP.max)
                        else:
                            nc.vector._custom_dve(
                                RK1MAX, out=t3s[:, u, :], in0=bc0a,
                                in1=bc0c, s0=bd0[:, t, 0:1],
                                s1=bd0[:, t, 1:2])
                    pts = vp.tile([128, 4, NB], BF16, tag="pts",
                                  name=f"pts0_{g}")
                    nc.vector.tensor_tensor(pts, t3s, mask_g[g],
                                            op=OP.mult)
                    # keep the PE clock gate warm through the VE loop
                    nc.tensor.matmul(psW, lhsT=ident, rhs=ident,
                                     start=True, stop=True)
                    nc.tensor.matmul(psW, lhsT=ident, rhs=ident,
                                     start=True, stop=True)
                    for u in range(4):
                        t = 4 * g + u
                        for s in range(IT):
                            nc.tensor.matmul(
                                ps0[s], lhsT=pts[:, u, ts(s, 128)],
                                rhs=r0q[t // NQ][:, t % NQ, 0:133],
                                start=(t == 0), stop=(t == JT - 1))

                # software-pipelined: build G quarter q+1 while the score
                # groups of quarter q run -- a strict per-quarter interleave
                # locksteps the engines (PE stalls on VE's pts before it can
                # start the next G quarter; VE stalls on PE's G PSUM)
                build_g_quarter(0)
                nc.tensor.matmul(
                    psW[:, 0:16], lhsT=ident,
                    rhs=bd0[:, 0:NQ, :].rearrange("p t c -> p (t c)"),
                    start=True, stop=True)
                for w in range(10):
                    nc.tensor.matmul(psW, lhsT=ident, rhs=ident,
                                     start=True, stop=True)
                for q in range(1, 4):
                    build_g_quarter(q)
                    for g in (2 * (q - 1), 2 * q - 1):
                        score_group0(g)
                for g in (6, 7):
                    score_group0(g)

            # ---------------- stage D: normalize + build G1 ----------------
            with tc.tile_pool(name="d_sb", bufs=2) as dsb, \
                 tc.tile_pool(name="d_ps", bufs=1, space="PSUM") as dps:

                rows12 = dsb.tile([1, 4, NB], BF16, tag="rows12", bufs=1)

                # one consolidated reciprocal for all 4 i-tiles (VE recip has
                # a ~1.2us fixed cost -- pay it once, not four times)
                den4 = dsb.tile([128, 4], F32, tag="den4", bufs=1)
                for s in range(IT):
                    nc.vector.tensor_copy(den4[:, s:s + 1],
                                          ps0[s][:, 132:133])
                r0c4 = dsb.tile([128, 4], F32, tag="r0c4", bufs=1)
                nc.vector.reciprocal(r0c4, den4)
                r0a4 = dsb.tile([128, 4], F32, tag="r0a4", bufs=1)
                nc.vector.tensor_scalar_mul(r0a4, r0c4, ALPHA)

                # f2' exps -> smallc columns [B1|D1|B2|D2], transpose to the
                # [4, NB] gather payload, and launch the SMALL AllGather
                # before the g1own builds so it absorbs the entry skew
                smallT = dsb.tile([4, NB], F32, tag="smallT", bufs=1)
                smallcs = []
                for s in range(IT):
                    r0c = r0c4[:, s:s + 1]
                    r0a = r0a4[:, s:s + 1]
                    smallc = dsb.tile([128, 4], F32, tag="smallc",
                                      name=f"smallc{s}", bufs=4)
                    nc.scalar.activation(smallc[:, 0:1], ps0[s][:, 129:130],
                                         AF.Exp, scale=r0c)
                    nc.scalar.activation(smallc[:, 1:2], ps0[s][:, 129:130],
                                         AF.Exp, scale=r0a)
                    nc.scalar.activation(smallc[:, 2:3], ps0[s][:, 131:132],
                                         AF.Exp, scale=r0c)
                    nc.scalar.activation(smallc[:, 3:4], ps0[s][:, 131:132],
                                         AF.Exp, scale=r0a)
                    smallcs.append(smallc)
                    psSm = dps.tile([4, 128], F32, tag="psT2")
                    nc.tensor.transpose(psSm, smallc, ident)
                    nc.vector.tensor_copy(smallT[:, ts(s, 128)], psSm)
                nc.sync.dma_start(out=sm_in, in_=smallT)
                nc.gpsimd.collective_compute(
                    "AllGather", OP.bypass, replica_groups=rg,
                    ins=[sm_in.opt()], outs=[sm_out.opt()])

                for s in range(IT):
                    r0c = r0c4[:, s:s + 1]
                    g1own = dsb.tile([128, G1W], BF16, tag="g1own")
                    nc.vector.tensor_scalar_mul(g1own[:, 0:64],
                                                ps0[s][:, 0:64], r0c)
                    nc.vector.memset(g1own[:, 64:65], 1.0)
                    nc.vector.tensor_copy(g1own[:, 65:67],
                                          smallcs[s][:, 0:2])
                    nc.vector.tensor_scalar_mul(g1own[:, 67:131],
                                                ps0[s][:, 64:128], r0c)
                    nc.vector.memset(g1own[:, 131:132], 1.0)
                    nc.vector.tensor_copy(g1own[:, 132:134],
                                          smallcs[s][:, 2:4])
                    nc.vector.memset(g1own[:, 134:136], 0.0)
                    nc.sync.dma_start(out=g1_in[ts(s, 128), :], in_=g1own)

                # small-gather payload back in; ALSO serializes the two
                # AllGathers: the gpsimd dep copy below keeps the big-AG
                # trigger behind the small AG's completion.  Two concurrent
                # in-flight collectives can cross-block on the CC cores
                # (one rank's ncfw picks up the big one first and blocks,
                # never serving the small one another rank waits on).
                sm_sb = dsb.tile([32, NB], F32, tag="sm_sb", bufs=1)
                nc.sync.dma_start(out=sm_sb, in_=sm_out)
                agdep = dsb.tile([1, 1], F32, tag="agdep", bufs=1)
                nc.gpsimd.tensor_copy(agdep, sm_sb[0:1, 0:1])
                nc.gpsimd.collective_compute(
                    "AllGather", OP.bypass, replica_groups=rg,
                    ins=[g1_in.opt()], outs=[g1_out.opt()])

                # f1' (col 128) and f1'' (col 130) -> exp'd rows; runs on
                # ACT/PE while the gathers are in flight
                for s in range(IT):
                    r0c = r0c4[:, s:s + 1]
                    for li, col in ((0, 128), (2, 130)):
                        fcl = dsb.tile([128, 1], F32, tag="fcl")
                        nc.scalar.activation(fcl, ps0[s][:, col:col + 1],
                                             AF.Copy, scale=r0c)
                        psT2 = dps.tile([1, 128], F32, tag="psT2")
                        nc.tensor.transpose(psT2, fcl, ident)
                        nc.scalar.activation(rows12[0:1, li, ts(s, 128)],
                                             psT2, AF.Exp)
                        nc.scalar.activation(rows12[0:1, li + 1, ts(s, 128)],
                                             psT2, AF.Exp, scale=ALPHA)

                for i, dst in enumerate((bc1a, bc1c, bc2a, bc2c)):
                    psB2 = dps.tile([128, NB], F32, tag="psB2")
                    nc.tensor.matmul(psB2, lhsT=ones1,
                                     rhs=rows12[0:1, i, :], start=True,
                                     stop=True)
                    nc.scalar.activation(dst, psB2, AF.Copy)

                # transpose the small-gather payload into bd12 layout
                for tr in range(4):
                    psBD = dps.tile([128, 32], F32, tag="psB2",
                                    name=f"psBD{tr}")
                    nc.tensor.transpose(psBD, sm_sb[:, ts(tr, 128)],
                                        ident[0:32, 0:32])
                    src = psBD.rearrange("p (r g) -> p r g", g=4)
                    for l in range(2):
                        dst = bd12[:, l, :, :].rearrange(
                            "p (r t4) c -> p t4 r c", t4=4)[:, tr]
                        nc.vector.tensor_copy(dst, src[:, :, 2 * l:2 * l + 2])

                for q in range(4):
                    nc.sync.dma_start(
                        out=r1q[q],
                        in_=g1_out[q * NQ * 128:(q + 1) * NQ * 128, :]
                        .rearrange("(t p) c -> p t c", p=128))

        # -------- stage E: layers 1+2, interleaved, transposed outputs -----
        # psT[c, i] = sum_j G1[j, c] * P[j, i]; row 64 = denominator.
        with tc.tile_pool(name="e_ps", bufs=1, space="PSUM") as eps, \
             tc.tile_pool(name="e_v", bufs=5) as vpl, \
             tc.tile_pool(name="e_sb", bufs=1) as esb:

            ps1T = eps.tile([65, NB], F32, tag="ps1T")
            ps2T = eps.tile([65, NB], F32, tag="ps2T")
            psW2 = eps.tile([128, 128], F32, tag="psW2")
            # HAM warm burst gated on the gathered G1 (the matmuls' actual
            # dependency) -- fires right as the big AllGather lands, while
            # the VE is still draining prebuilt score tiles
            nc.tensor.matmul(psW2, lhsT=r1q[0][:, 0, 0:128],
                             rhs=bc1a[:, 0:128], start=True, stop=True)
            for w in range(16):
                nc.tensor.matmul(psW2, lhsT=ident, rhs=ident,
                                 start=True, stop=True)

            def p_group(g, uniq, bca, bcc, bd, pool):
                # deep pts/t3s rings: score tiles only need bd12 (small
                # gather) + local bcasts + mask, so the VE prebuilds them
                # while the big G1 AllGather is still in flight
                t3s = pool.tile([128, 4, NB], BF16, tag="t3s", bufs=4,
                                name=f"t3se_{uniq}_{g}")
                for u in range(4):
                    t = 4 * g + u
                    if (t + 2 * uniq) % 15 < 8:
                        t1 = pool.tile([128, NB], BF16, tag="t1", bufs=6,
                                       name=f"t1e_{uniq}_{t}")
                        nc.scalar.activation(t1, bca, AF.Copy,
                                             scale=bd[:, t, 0:1])
                        t2 = pool.tile([128, NB], BF16, tag="t2", bufs=6,
                                       name=f"t2e_{uniq}_{t}")
                        nc.scalar.activation(t2, bcc, AF.Copy,
                                             scale=bd[:, t, 1:2])
                        nc.vector.tensor_tensor(t3s[:, u, :], t1, t2,
                                                op=OP.max)
                    else:
                        nc.vector._custom_dve(
                            RK1MAX, out=t3s[:, u, :], in0=bca, in1=bcc,
                            s0=bd[:, t, 0:1], s1=bd[:, t, 1:2])
                pts = pool.tile([128, 4, NB], BF16, tag="pts", bufs=16,
                                name=f"ptse_{uniq}_{g}")
                nc.vector.tensor_tensor(pts, t3s, mask_g[g], op=OP.mult)
                # keep the PE clock-gate warm through the VE-paced drain
                nc.tensor.matmul(psW2, lhsT=ident, rhs=ident,
                                 start=True, stop=True)
                nc.tensor.matmul(psW2, lhsT=ident, rhs=ident,
                                 start=True, stop=True)
                return pts

            # pass 2 (logstd) first so its Z-chain overlaps pass 1
            for g in range(JT // 4):
                pts = p_group(g, 2, bc2a, bc2c, bd12[:, 1, :, :], vpl)
                for u in range(4):
                    t = 4 * g + u
                    nc.tensor.matmul(ps2T,
                                     lhsT=r1q[t // NQ][:, t % NQ, 67:132],
                                     rhs=pts[:, u, :],
                                     start=(t == 0), stop=(t == JT - 1))

            r2row = esb.tile([1, NB], F32)
            nc.vector.reciprocal(r2row, ps2T[64:65, :])
            r2r = esb.tile([1, NB], F32R)
            nc.scalar.activation(r2r, r2row, AF.Copy)
            psBC2 = eps.tile([64, NB], F32, tag="psBC2")
            nc.tensor.matmul(psBC2, lhsT=onesr, rhs=r2r, start=True,
                             stop=True)
            r2bc = esb.tile([64, NB], F32)
            nc.scalar.activation(r2bc, psBC2, AF.Copy)
            ltT = esb.tile([64, NB], F32)
            nc.vector.tensor_tensor(ltT, ps2T[0:64, :], r2bc, op=OP.mult)
            eT = esb.tile([64, NB], F32)
            nc.scalar.activation(eT, ltT, AF.Exp)
            zmT = esb.tile([64, NB], F32)
            nc.vector.tensor_tensor(zmT, eT, noiseT_sb, op=OP.mult)

            for g in range(JT // 4):
                pts = p_group(g, 1, bc1a, bc1c, bd12[:, 0, :, :], vpl)
                for u in range(4):
                    t = 4 * g + u
                    nc.tensor.matmul(ps1T,
                                     lhsT=r1q[t // NQ][:, t % NQ, 0:65],
                                     rhs=pts[:, u, :],
                                     start=(t == 0), stop=(t == JT - 1))

            r1row = esb.tile([1, NB], F32)
            nc.vector.reciprocal(r1row, ps1T[64:65, :])
            r1r = esb.tile([1, NB], F32R)
            nc.scalar.activation(r1r, r1row, AF.Copy)
            psBC1 = eps.tile([64, NB], F32, tag="psBC1")
            nc.tensor.matmul(psBC1, lhsT=onesr, rhs=r1r, start=True,
                             stop=True)
            r1bc = esb.tile([64, NB], F32)
            nc.scalar.activation(r1bc, psBC1, AF.Copy)
            meanT = esb.tile([64, NB], F32)
            nc.vector.tensor_tensor(meanT, ps1T[0:64, :], r1bc, op=OP.mult)
            zT = esb.tile([64, NB], F32)
            nc.vector.tensor_tensor(zT, zmT, meanT, op=OP.add)
            nc.scalar.activation(zt_own, zT, AF.Copy)

        # ---------------- stage F: gather Z^T -----------------------------
        nc.sync.dma_start(out=ztg_in, in_=zt_own)
        nc.gpsimd.collective_compute(
            "AllGather", OP.bypass, replica_groups=rg,
            ins=[ztg_in.opt()], outs=[ztg_out.opt()])
        # split the gathered Z^T load per block pair so the first decoder
        # matmuls start ~1us after the gather instead of waiting the full
        # 512KB strided load
        for bp in range(NCORES // 2):
            nc.sync.dma_start(
                out=ztb[:, 2 * bp:2 * bp + 2, :],
                in_=ztg_out[2 * bp * 64:(2 * bp + 2) * 64, :]
                .rearrange("(b p) i -> p b i", p=64))

        # ---------------- stage G: decoder sigmoid(Z @ Z^T) ----------------
        with tc.tile_pool(name="dec_ps", bufs=3, space="PSUM") as decps, \
             tc.tile_pool(name="dec_sb", bufs=3) as decsb:
            # own (diagonal) blocks first — pure warm-up compute that
            # overlaps the Z^T gather (its output position would be
            # core-dependent, so the result is discarded; the paired loop
            # recomputes it)
            for s in range(IT):
                psD = decps.tile([128, NB], F32, tag="psDd",
                                 name=f"psDd_{s}", bufs=1)
                nc.tensor.matmul(psD, lhsT=zt_own[:, ts(s, 128)],
                                 rhs=zt_own, start=True, stop=True)
                osb = decsb.tile([128, NB], F16, tag="osbd",
                                 name=f"osbd_{s}", bufs=1)
                nc.scalar.activation(osb, psD, AF.Sigmoid)

            # PE warm burst during the rest of the Z^T gather window (local
            # data only; the gather window is otherwise PE-idle)
            psWd = decps.tile([128, NB], F32, tag="psWd", bufs=1)
            nc.tensor.matmul(psWd, lhsT=zt_own[:, 0:128], rhs=zt_own,
                             start=True, stop=True)
            for w in range(12):
                nc.tensor.matmul(psWd[:, 0:128], lhsT=ident, rhs=ident,
                                 start=True, stop=True)

            # paired j-blocks: 2 matmuls into one 2-bank PSUM tile, then a
            # single [128, 1024] sigmoid and a single contiguous DMA out.
            # bp-outer order matches the split ztb DMA arrival order.
            for bp in range(NCORES // 2):
                for s in range(IT):
                    psD2 = decps.tile([128, 2, NB], F32, tag="psD2",
                                      name=f"psD2_{s}_{bp}", bufs=3)
                    nc.tensor.matmul(psD2[:, 0, :],
                                     lhsT=zt_own[:, ts(s, 128)],
                                     rhs=ztb[:, 2 * bp, :],
                                     start=True, stop=True)
                    nc.tensor.matmul(psD2[:, 1, :],
                                     lhsT=zt_own[:, ts(s, 128)],
                                     rhs=ztb[:, 2 * bp + 1, :],
                                     start=True, stop=True)
                    osb2 = decsb.tile([128, 2, NB], F16, tag="osb2",
                                      name=f"osb2_{s}_{bp}", bufs=6)
                    nc.scalar.activation(osb2, psD2, AF.Sigmoid)
                    eng = nc.sync if bp % 2 == 0 else nc.gpsimd
                    eng.dma_start(
                        out=apred[ts(s, 128), ts(bp, 2 * NB)], in_=osb2)

    nc.compile()
    return nc


_program = None


def _get_program():
    global _program
    if _program is None:
        _program = build_program()
    return _program


def kernel(X, adj, noise, W0, a0, W1, a1, W2, a2, _trace=False):
    X = np.asarray(X, dtype=np.float32)
    adj = np.asarray(adj)
    noise = np.asarray(noise, dtype=np.float32)
    W0 = np.asarray(W0, dtype=np.float32)
    a0 = np.asarray(a0, dtype=np.float32)
    W1 = np.asarray(W1, dtype=np.float32)
    a1 = np.asarray(a1, dtype=np.float32)
    W2 = np.asarray(W2, dtype=np.float32)
    a2 = np.asarray(a2, dtype=np.float32)

    # folded weight matrix [512, 133]: attention rhs cols + f2 weight
    u1 = W0 @ W1
    u2 = W0 @ W2
    wbig = np.concatenate([
        u1, u2,
        u1 @ a1[:H2], u1 @ a1[H2:],
        u2 @ a2[:H2], u2 @ a2[H2:],
        W0 @ a0[H1:],
    ], axis=1).astype(np.float32)
    wfo = (W0 @ a0[:H1]).astype(np.float32)  # [512, 1] f1 weight

    maskT = adj.astype(ml_dtypes.bfloat16).T  # 0/1, exact in bf16

    def rearr(m):
        # [512, c] -> [128, 4, c] matching the device-side k-split
        c = m.shape[1]
        return np.ascontiguousarray(
            m.reshape(4, 128, c).transpose(1, 0, 2)).astype(np.float16)

    xt_full = rearr(np.ascontiguousarray(X.T))     # [128, 4, 4096], shared
    wbig_r = rearr(wbig)
    wfo_r = rearr(wfo)

    in_maps = []
    for k in range(NCORES):
        sl = slice(k * NB, (k + 1) * NB)
        in_maps.append({
            "xt": xt_full,
            "wbig": wbig_r,
            "xto": rearr(np.ascontiguousarray(X[sl].T)),
            "wfo": wfo_r,
            "maskT": np.ascontiguousarray(maskT[:, sl]),
            "noiseT": np.ascontiguousarray(noise[sl].T),
        })

    nc = _get_program()
    res = run_bass_kernel_spmd(nc, in_maps, core_ids=list(range(NCORES)),
                               trace=_trace)
    out = np.concatenate(
        [res.results[k]["apred"].astype(np.float32) for k in range(NCORES)],
        axis=0)
    if _trace:
        kernel.last_results = res
    return out

